# revision 39
# baseline (speedup 1.0000x reference)
"""Trainium2 Bass kernel for nn_KANModel (KAN recommender).

Math: with a shared uniform grid (G=5, k=3), the cubic B-spline bases on the
extended uniform knots are shifted cardinal splines, so each KAN layer is a
sum of relu(u-n)^3 maps plus a silu path. Two exact reductions applied here:

1. Centered-poly split: blocks whose hinge n lies at or below the layer's
   input-interval minimum have relu(u-n) == u-n identically, so their sum
   collapses into ONE cubic polynomial evaluated in v = u - c (centered to
   keep fp magnitudes ~1): three matmuls (v, v^2, v^3) + a constant folded
   into the bias. Only hinges strictly inside the interval stay as genuine
   relu blocks (interval bounds: exact table extrema for layer 0, rigorous
   grid+Lipschitz bounds for layer 1).
2. Feature-major everywhere: layer-0 output is accumulated directly as
   hT[o, b] (weights stationary), so layer 1 needs no transpose; its
   contraction runs as out-free=1 matmuls (lhsT = activation tiles).

Embedding tables are host-concatenated (item indices get +NU) and stored
fp16; two indirect gathers (one per table row lookup) feed two PE
transposes into a feature-major xT. All PE matmuls run as float32r
(bitcast) for 2x PE throughput. Activations use only {Sigmoid, Square}
(one table set -> single table load). Data-parallel over batch:
1024 rows -> 8 cores x 128.
"""

import numpy as np

B_FULL = 1024
NCORES = 8
BS = B_FULL // NCORES          # batch shard per core
D = 64                         # embedding dim
IN0, OUT0 = 2 * D, 64          # KAN layer 0
IN1 = 64                       # KAN layer 1 (out_dim 1)
G, KORD = 5, 3
NC_BASIS = G + KORD            # 8 spline bases per edge
NZ = G + 2 * KORD + 1          # 12 possible relu-cube shifts
NU, NI = 100000, 50000

_BUILD_CACHE = {}
TRACE = False
LAST_RESULTS = None

_A5 = np.array([1.0, -4.0, 6.0, -4.0, 1.0], dtype=np.float64) / 6.0
_C3 = np.array([1.0, 3.0, 3.0, 1.0], dtype=np.float64)  # C(3, j)


def _m3(s):
    """Cardinal cubic B-spline, exact (clamped) evaluation, float64."""
    s = np.minimum(s, 4.0)
    out = np.zeros_like(s)
    for m in range(4):
        r = np.maximum(s - m, 0.0)
        out += _A5[m] * r * r * r
    return out


def _fold_host_weights(grid0, coef0, sb0, ssp0, bias0, grid1, coef1, sb1, ssp1,
                       bias1, x_min, x_max):
    """O(params) host-side prep: folded weights with centered-poly split."""
    h0 = float(grid0[0, -1] - grid0[0, 0]) / G
    t0_0 = float(grid0[0, 0]) - KORD * h0
    h1 = float(grid1[0, -1] - grid1[0, 0]) / G
    t0_1 = float(grid1[0, 0]) - KORD * h1

    # ---- layer 0: exact interval from table extrema ----
    u0_min = (x_min - t0_0) / h0
    u0_max = (x_max - t0_0) / h0
    nlist0 = [n for n in range(NZ) if n < u0_max + 1e-6]
    poly0 = [n for n in nlist0 if n <= u0_min]
    rel0 = [n for n in nlist0 if n > u0_min]
    c0 = 0.5 * (u0_min + u0_max)

    c0e = (ssp0[:, None].astype(np.float64) * coef0.astype(np.float64)).reshape(
        OUT0, IN0, NC_BASIS
    )  # (o, f, c)

    def wz0_of(n):
        w = np.zeros((IN0, OUT0), dtype=np.float64)
        for m in range(5):
            c = n - m
            if 0 <= c < NC_BASIS:
                w += _A5[m] * c0e[:, :, c].T
        return w  # [f, o]

    # raw-x form: w_n*relu(u-n)^3 = (w_n/h0^3)*relu(x-g_n)^3, g_n = t0+n*h0.
    # poly blocks collapse to a cubic in raw x (|x| <= ~0.55, well-centered)
    inv_h0c = (1.0 / h0) ** 3
    P0 = np.zeros((4, IN0, OUT0), dtype=np.float64)
    for n in poly0:
        w = wz0_of(n) * inv_h0c
        g = t0_0 + n * h0
        for j in range(4):
            P0[j] += w * (_C3[j] * (-g) ** (3 - j))
    b0p = bias0.astype(np.float64) + P0[0].sum(axis=0)  # [o]
    wz0r = [wz0_of(n) * inv_h0c for n in rel0]

    sb0e = sb0.reshape(OUT0, IN0).astype(np.float64)  # (o, f)

    # ---- rigorous layer-0 output interval (grid + Lipschitz pad) ----
    NGRID = 2049
    xg = np.linspace(x_min, x_max, NGRID)
    dx = (x_max - x_min) / (NGRID - 1) if x_max > x_min else 0.0
    ug = (xg - t0_0) / h0
    basis = np.stack([_m3(ug - c) for c in range(NC_BASIS)], axis=1)  # (g, c)
    silug = xg / (1.0 + np.exp(-xg))
    phi = sb0e[:, :, None] * silug[None, None, :] + np.einsum(
        "ofc,gc->ofg", c0e, basis
    )
    lip = np.abs(sb0e) * 1.1 + np.abs(c0e).sum(axis=2) * (0.75 / h0)
    pad = lip * dx
    h_min = bias0.astype(np.float64) + (phi.min(axis=2) - pad).sum(axis=1)
    h_max = bias0.astype(np.float64) + (phi.max(axis=2) + pad).sum(axis=1)
    # 0.05 u-units of one-time margin for device fp error in h
    u1_min = (float(h_min.min()) - t0_1) / h1 - 0.05
    u1_max = (float(h_max.max()) - t0_1) / h1 + 0.05
    nlist1 = [n for n in range(NZ) if n < u1_max + 1e-3]
    poly1 = [n for n in nlist1 if n <= u1_min]
    rel1 = [n for n in nlist1 if n > u1_min]
    c1 = 0.5 * (u1_min + u1_max)

    c1e = ssp1[:, None].astype(np.float64) * coef1.astype(np.float64)  # (64, 8)

    def wz1_of(n):
        w = np.zeros((IN1,), dtype=np.float64)
        for m in range(5):
            c = n - m
            if 0 <= c < NC_BASIS:
                w += _A5[m] * c1e[:, c]
        return w  # [i]

    inv_h1c = (1.0 / h1) ** 3
    P1 = np.zeros((4, IN1), dtype=np.float64)
    for n in poly1:
        w = wz1_of(n) * inv_h1c
        g = t0_1 + n * h1
        for j in range(4):
            P1[j] += w * (_C3[j] * (-g) ** (3 - j))
    b_final = float(bias1[0]) + float(P1[0].sum())
    wz1r = [wz1_of(n) * inv_h1c for n in rel1]

    # pair genuine layer-1 blocks into [128]-partition tiles
    G1 = len(rel1)
    P1pairs = (G1 + 1) // 2
    nvals = np.zeros((128, max(P1pairs, 1)), dtype=np.float64)
    w1p = np.zeros((128, max(P1pairs, 1)), dtype=np.float64)
    for k in range(P1pairs):
        na = rel1[2 * k]
        nvals[0:64, k] = t0_1 + na * h1
        w1p[0:64, k] = wz1r[2 * k]
        if 2 * k + 1 < G1:
            nb = rel1[2 * k + 1]
            nvals[64:128, k] = t0_1 + nb * h1
            w1p[64:128, k] = wz1r[2 * k + 1]
        else:
            nvals[64:128, k] = 1e6  # relu always 0
            w1p[64:128, k] = 0.0

    # ---- pack the single weight blob [128, W] ----
    G0 = len(rel0)
    cols = {}
    pieces = []
    c = 0

    def add(name, arr128):
        nonlocal c
        w = arr128.shape[1]
        cols[name] = c
        pieces.append(arr128.astype(np.float32))
        c += w

    add("sb0w", sb0e.T)                        # [128, 64]
    add("P01", P0[1])
    add("P02", P0[2])
    add("P03", P0[3])
    for g in range(G0):
        add(f"wz0_{g}", wz0r[g])
    b0row = np.zeros((128, 64))
    b0row[0, :] = b0p
    add("b0row", b0row)
    vr = np.zeros((128, 4))
    vr[0:64, 0] = P1[1]
    vr[0:64, 1] = P1[2]
    vr[0:64, 2] = P1[3]
    vr[0:64, 3] = sb1.astype(np.float64)
    add("vrhs", vr)
    add("nvals", nvals)
    add("w1p", w1p)
    onesrow = np.zeros((128, BS))
    onesrow[0, :] = 1.0
    add("ones", onesrow)
    sidx16 = np.zeros((16, 8), dtype=np.int16)
    for p in range(16):
        for s in range(8):
            sidx16[p, s] = s * 16 + p
    sidx = np.zeros((128, 4), dtype=np.float32)
    sidx[0:16, :] = sidx16.view(np.float32)
    add("sidx", sidx)
    blob = np.ascontiguousarray(np.concatenate(pieces, axis=1).astype(np.float32))

    consts = (
        0.0, 1.0, 0.0, 1.0, b_final,
        tuple(float(t0_0 + n * h0) for n in rel0), G1, P1pairs,
        len(poly1) > 0,
        tuple(sorted(cols.items())), blob.shape[1],
    )
    return consts, blob, cols


def _build_program(consts):
    import concourse.bass as bass
    import concourse.bacc as bacc
    import concourse.mybir as mybir
    from concourse.tile import TileContext
    from concourse.masks import make_identity

    (xc0, inv_h0, xc1, inv_h1, b_final, rel0_off, G1, P1pairs, has_poly1,
     cols_t, W) = consts
    cols = dict(cols_t)
    G0 = len(rel0_off)
    NB0 = 1 + G0                   # v0 + genuine relu blocks (layer 0)
    f32 = mybir.dt.float32
    f16 = mybir.dt.float16
    i32 = mybir.dt.int32
    i16 = mybir.dt.int16
    A = mybir.AluOpType
    AF = mybir.ActivationFunctionType
    NTAB = NU + NI

    nc = bacc.Bacc("TRN2")
    d_idx = nc.dram_tensor("idx", [BS, 2], i32, kind="ExternalInput")
    d_emb = nc.dram_tensor("emb", [NTAB, D], f16, kind="ExternalInput")
    d_blob = nc.dram_tensor("blob", [128, W], f32, kind="ExternalInput")
    d_out = nc.dram_tensor("out", [BS, 64], f32, kind="ExternalOutput")

    with TileContext(nc) as tc:
        with (
            tc.tile_pool(name="sb", bufs=1) as P,
            tc.tile_pool(name="ps", bufs=1, space="PSUM") as PS,
        ):
            idx = P.tile([BS, 2], i32, tag="idx")
            nc.sync.dma_start(out=idx[:], in_=d_idx[:])
            blob = P.tile([128, W], f32, tag="blob")
            nc.sync.dma_start(out=blob[:], in_=d_blob[:])

            def wtile(name, w):
                c = cols[name]
                return blob[:, c : c + w]

            ident = P.tile([128, 128], f32, tag="ident")
            make_identity(nc, ident[:])
            zt = P.tile([BS, 64], f32, tag="zt")
            nc.gpsimd.memset(zt[:, :], 0.0)
            nc.sync.dma_start(out=d_out[:], in_=zt[:, :])
            osb = P.tile([BS, 1], f32, tag="osb")

            # gather embeddings (fp16 rows, one row per batch partition):
            # user dims -> cols 0:64, item dims -> cols 64:128
            xg = P.tile([BS, 2 * D], f16, tag="xg")
            nc.gpsimd.indirect_dma_start(
                out=xg[:, 0:D], out_offset=None, in_=d_emb[:],
                in_offset=bass.IndirectOffsetOnAxis(ap=idx[:, 0:1], axis=0),
            )
            nc.gpsimd.indirect_dma_start(
                out=xg[:, D : 2 * D], out_offset=None, in_=d_emb[:],
                in_offset=bass.IndirectOffsetOnAxis(ap=idx[:, 1:2], axis=0),
            )

            # prepared output scatter: descriptors generated now (Pool is
            # idle), fired by trigger_dma once osb is written
            dma_sem = nc.alloc_semaphore("out_scatter_dma")
            nc.gpsimd.dma_scatter_add(
                out_ap=d_out[:, 0:1], in_ap=osb[:, 0:1],
                idxs_ap=wtile("sidx", 4)[0:16, :].bitcast(i16),
                num_idxs=BS, num_idxs_reg=BS, elem_size=1, elem_step=64,
                prepare_only=True, sem=dma_sem,
            )

            # fp16 -> f32 converts (fp16 transposes would trigger standalone
            # Ldweights in tile-legalize, which walrus rejects here)
            xbm = P.tile([BS, 2 * D], f32, tag="xbm")
            nc.vector.tensor_copy(out=xbm[:, 0:D], in_=xg[:, 0:D])
            nc.vector.tensor_copy(out=xbm[:, D : 2 * D], in_=xg[:, D : 2 * D])
            # transpose to feature-major xT: (f, b)
            xT = PS.tile([128, BS], f32, tag="xT")
            nc.tensor.matmul(out=xT[:], lhsT=xbm[:], rhs=ident[:],
                             is_transpose=True, start=True, stop=True)

            # ---- layer 0 (batch-major accumulation hps[b, o]) ----
            vq = P.tile([128, NB0 * BS], f32, tag="vq")
            qq = P.tile([128, NB0 * BS], f32, tag="qq")
            zq = P.tile([128, NB0 * BS], f32, tag="zq")
            nc.vector.tensor_copy(out=vq[:, 0:BS], in_=xT[:])
            for g, noff in enumerate(rel0_off):
                nc.vector.tensor_scalar(
                    vq[:, (1 + g) * BS : (2 + g) * BS], vq[:, 0:BS],
                    float(noff), 0.0, A.subtract, A.max,
                )
            # ACT: sigmoid first (needs only xT), then squares with the
            # relu blocks (tail-critical) ahead of the v0 block
            sg0 = P.tile([128, BS], f32, tag="sg0")
            nc.scalar.activation(sg0[:], xT[:], AF.Sigmoid)
            if G0 > 0:
                nc.scalar.activation(qq[:, BS : NB0 * BS], vq[:, BS : NB0 * BS],
                                     AF.Square)
            nc.scalar.activation(qq[:, 0:BS], vq[:, 0:BS], AF.Square)
            silu0 = P.tile([128, BS], f32, tag="silu0")
            nc.gpsimd.tensor_tensor(out=silu0[:], in0=sg0[:],
                                    in1=vq[:, 0:BS], op=A.mult)
            # cubes: relu blocks batched on DVE (tail-critical), then v0
            if G0 > 0:
                nc.vector.tensor_tensor(out=zq[:, BS : NB0 * BS],
                                        in0=qq[:, BS : NB0 * BS],
                                        in1=vq[:, BS : NB0 * BS], op=A.mult)
            nc.vector.tensor_tensor(out=zq[:, 0:BS], in0=qq[:, 0:BS],
                                    in1=vq[:, 0:BS], op=A.mult)

            hps = PS.tile([BS, OUT0], f32, tag="hps")

            def mmw(lhs_ap, rhs_ap, start, stop):
                nc.tensor.matmul(out=hps[:], lhsT=lhs_ap, rhs=rhs_ap,
                                 start=start, stop=stop)

            mmw(wtile("ones", BS)[0:1, :], wtile("b0row", 64)[0:1, :],
                True, False)
            mmw(vq[:, 0:BS], wtile("P01", 64), False, False)
            mmw(silu0[:], wtile("sb0w", 64), False, False)
            mmw(qq[:, 0:BS], wtile("P02", 64), False, False)
            for g in range(G0):
                mmw(zq[:, (1 + g) * BS : (2 + g) * BS], wtile(f"wz0_{g}", 64),
                    False, False)
            mmw(zq[:, 0:BS], wtile("P03", 64), False, True)

            # ---- transpose h to feature-major, duplicated into both
            # partition halves: hsb2 = [h | h] so hsb2^T = [hT; hT] ----
            hsb2 = P.tile([BS, 2 * OUT0], f32, tag="hsb2")
            nc.vector.tensor_copy(out=hsb2[:, 0:OUT0], in_=hps[:])
            nc.vector.tensor_copy(out=hsb2[:, OUT0 : 2 * OUT0], in_=hps[:])
            hT2 = PS.tile([128, BS], f32, tag="hT2")
            nc.tensor.matmul(out=hT2[:], lhsT=hsb2[:], rhs=ident[:],
                             is_transpose=True, start=True, stop=True)

            # ---- layer 1 (feature-major, out-free=1 matmuls into y) ----
            NP = max(P1pairs, 1)
            v2 = P.tile([128, BS], f32, tag="v2")
            nc.vector.tensor_copy(out=v2[:, :], in_=hT2[:])
            sg1 = P.tile([64, BS], f32, tag="sg1")
            nc.scalar.activation(sg1[:], hT2[0:64, :], AF.Sigmoid)

            y = PS.tile([BS, 1], f32, tag="y")

            def mm1(lhs_ap, rhs_ap, start, stop):
                nc.tensor.matmul(out=y[:], lhsT=lhs_ap, rhs=rhs_ap,
                                 start=start, stop=stop)

            vrhs = wtile("vrhs", 4)
            r1 = P.tile([128, NP * BS], f32, tag="r1")
            q1 = P.tile([128, NP * BS], f32, tag="q1")
            z1 = P.tile([128, NP * BS], f32, tag="z1")
            # r blocks: DVE (fast) with Pool helping on the late pairs
            for k in range(P1pairs):
                eng = nc.vector if k < (P1pairs + 1) // 2 else nc.gpsimd
                eng.tensor_scalar(
                    r1[:, k * BS : (k + 1) * BS], v2[:],
                    wtile("nvals", NP)[:, k : k + 1], 0.0, A.subtract, A.max,
                )
            silu1 = P.tile([64, BS], f32, tag="silu1")
            nc.gpsimd.tensor_tensor(out=silu1[:], in0=sg1[:],
                                    in1=v2[0:64, :], op=A.mult)
            mm1(silu1[:], vrhs[0:64, 3:4], True, False)
            if has_poly1:
                sqv = P.tile([64, BS], f32, tag="sqv")
                nc.scalar.activation(sqv[:], v2[0:64, :], AF.Square)
                cbv = P.tile([64, BS], f32, tag="cbv")
                nc.gpsimd.tensor_tensor(out=cbv[:], in0=sqv[:],
                                        in1=v2[0:64, :], op=A.mult)
                mm1(v2[0:64, :], vrhs[0:64, 0:1], False, False)
                mm1(sqv[:], vrhs[0:64, 1:2], False, False)
                mm1(cbv[:], vrhs[0:64, 2:3], False, False)
            # chunked square -> paired cube -> matmul pipeline
            for k0 in range(0, P1pairs, 2):
                k1 = min(k0 + 2, P1pairs)
                sl = slice(k0 * BS, k1 * BS)
                nc.scalar.activation(q1[:, sl], r1[:, sl], AF.Square)
                nc.vector.tensor_tensor(out=z1[:, sl], in0=q1[:, sl],
                                        in1=r1[:, sl], op=A.mult)
                for k in range(k0, k1):
                    mm1(z1[:, k * BS : (k + 1) * BS],
                        wtile("w1p", NP)[:, k : k + 1], False,
                        k == P1pairs - 1)

            nc.scalar.activation(osb[:], y[:], AF.Sigmoid, bias=float(b_final))
            nc.gpsimd.trigger_dma(count=None)

    nc.compile()
    # Repoint the scatter-prep's DMA-completion update at the Tile-assigned
    # DMASW lane sem: tile epilogue waits on that lane (>= 16), but
    # dma_scatter_add bakes the user-provided sem into on_update[0], which
    # the trigger path fires instead -- leaving the lane sem stuck at 0.
    upd = set()
    waited = {}
    prep = None
    for b in nc.main_func.blocks:
        for ins in b.instructions:
            si = ins.sync_info
            if si is None:
                continue
            for u in si.on_update or []:
                if (u.ant_name or "").startswith("DMASW"):
                    upd.add(u.id)
            for w in si.on_wait or []:
                if (w.ant_name or "").startswith("DMASW"):
                    waited[w.id] = w.ant_name
            if type(ins).__name__ == "InstDMAScatterAddAnt":
                prep = ins
    missing = sorted(i for i in waited if i not in upd)
    if prep is not None and missing:
        u0 = prep.sync_info.on_update[0]
        u0.id = missing[0]
        u0.ant_name = waited[missing[0]]
        # The scatter's payload is in DRAM once its transfer completes; the
        # +900ns sem-propagation only matters for on-device consumers, of
        # which there are none. Drop the epilogue's wait on that lane so the
        # end-of-program barrier isn't serialized behind the propagation.
        trig = None
        for b in nc.main_func.blocks:
            for ins in b.instructions:
                if type(ins).__name__ == "InstTriggerDma":
                    trig = ins
        drop_ids = set(missing)
        if trig is not None and trig.sync_info and trig.sync_info.on_update:
            tu = trig.sync_info.on_update[0]
            waiters = 0
            for b in nc.main_func.blocks:
                for ins in b.instructions:
                    si = ins.sync_info
                    if si and si.on_wait and any(w.id == tu.id for w in si.on_wait):
                        waiters += 1
            if waiters == 1:
                drop_ids.add(tu.id)
                trig.sync_info = mybir.SyncInfo(
                    on_wait=list(trig.sync_info.on_wait), on_update=[])
        for b in nc.main_func.blocks:
            for ins in b.instructions:
                si = ins.sync_info
                if si is None or not si.on_wait:
                    continue
                if any(w.id in drop_ids for w in si.on_wait):
                    keep = [w for w in si.on_wait if w.id not in drop_ids]
                    ins.sync_info = mybir.SyncInfo(
                        on_wait=keep, on_update=list(si.on_update or []))
    return nc


def kernel(
    user_indices, item_indices, grid_update_num, stop_grid_update_step,
    emb_user, emb_item,
    grid0, coef0, sb0, ssp0, bias0,
    grid1, coef1, sb1, ssp1, bias1,
):
    global LAST_RESULTS
    from concourse.bass_utils import run_bass_kernel_spmd

    uidx = np.asarray(user_indices).astype(np.int32).reshape(B_FULL, 1)
    iidx = np.asarray(item_indices).astype(np.int32).reshape(B_FULL, 1) + NU
    emb = np.ascontiguousarray(
        np.concatenate(
            [np.asarray(emb_user, dtype=np.float32),
             np.asarray(emb_item, dtype=np.float32)], axis=0
        ).astype(np.float16)
    )
    embf = emb.astype(np.float64)
    x_min = float(embf.min())
    x_max = float(embf.max())

    consts, blob, _cols = _fold_host_weights(
        np.asarray(grid0, dtype=np.float32), np.asarray(coef0, dtype=np.float32),
        np.asarray(sb0, dtype=np.float32), np.asarray(ssp0, dtype=np.float32),
        np.asarray(bias0, dtype=np.float32), np.asarray(grid1, dtype=np.float32),
        np.asarray(coef1, dtype=np.float32), np.asarray(sb1, dtype=np.float32),
        np.asarray(ssp1, dtype=np.float32), np.asarray(bias1, dtype=np.float32),
        x_min, x_max,
    )

    key = consts[:9] + (consts[10],)
    if key not in _BUILD_CACHE:
        _BUILD_CACHE[key] = _build_program(consts)
    nc = _BUILD_CACHE[key]

    in_maps = []
    for c in range(NCORES):
        sl = slice(c * BS, (c + 1) * BS)
        in_maps.append(
            {
                "idx": np.ascontiguousarray(
                    np.concatenate([uidx[sl], iidx[sl]], axis=1)),
                "emb": emb,
                "blob": blob,
            }
        )

    res = run_bass_kernel_spmd(nc, in_maps, core_ids=list(range(NCORES)),
                               trace=TRACE)
    LAST_RESULTS = res
    return np.concatenate([r["out"][:, 0:1] for r in res.results], axis=0)
